# revision 9
# baseline (speedup 1.0000x reference)
"""Trainium2 Bass kernel for nn_HRMReasoning (8-core data parallel).

Key math: stack_pass is affine (z -> z @ W.T + b composed 6x), so every
segment's L-part (15 stack passes) and H-part (3 stack passes) collapse to
single affine maps; segment t's cumulative map is the t-th power. The ACT
halting trajectory only needs q_t = sigmoid(zh_t @ q_w.T + q_b) where
zh_t = zh_0 @ (P^t).T + d_t, so all 11 segment logits come from ONE matmul
against a folded [256, 22] matrix. One AllGather replaces 11 AllReduces.
The final state is selected by the halting index m via an indirect-DMA
gather from a precomposed power table, then applied with 2 matmuls.

Sharding: batch/env dim block-sharded across 8 cores; each core gets its
512-row slice of the carries and local env ids (hint-sanctioned pure data
parallel; env ids are assumed to resolve within the core's block).
"""

import numpy as np

EMBED = 256
NUM_LAYERS = 6
H_CYCLES = 3
L_CYCLES = 5
MMIN = 1
MMAX = 10
T = MMAX + 1          # 11 segments max
B = 4096
N_CORES = 8
BP = B // N_CORES     # 512 rows per core
RT = BP // 128        # 4 row-tiles per core
BLK = 2 * EMBED + 2   # 514 rows per segment block in the power table


def _compose_stack(W, bvec):
    """Affine map M, c with stack_pass(z) == z @ M.T + c (float64)."""
    M = np.eye(EMBED, dtype=np.float64)
    c = np.zeros(EMBED, dtype=np.float64)
    for i in range(NUM_LAYERS):
        Wi = W[i].astype(np.float64)
        M = Wi @ M
        c = Wi @ c + bvec[i].astype(np.float64)
    return M, c


def _compose_pow(M, c, n):
    Mn = np.eye(EMBED, dtype=np.float64)
    cn = np.zeros(EMBED, dtype=np.float64)
    for _ in range(n):
        cn = M @ cn + c
        Mn = M @ Mn
    return Mn, cn


def _host_consts(L_w, L_b, H_w, H_b, q_w, q_b):
    ML, cL = _compose_stack(L_w, L_b)
    MH, cH = _compose_stack(H_w, H_b)
    MLs, cLs = _compose_pow(ML, cL, 15)   # one segment of L
    MHs, cHs = _compose_pow(MH, cH, 3)    # one segment of H

    q_w64 = q_w.astype(np.float64)
    q_b64 = q_b.astype(np.float64)

    stack = np.zeros((T * BLK, EMBED), np.float32)
    GT = np.zeros((EMBED, 2 * T), np.float32)
    grow = np.zeros((1, 2 * T), np.float32)

    Mcur = np.eye(EMBED); ccur = np.zeros(EMBED)
    Pcur = np.eye(EMBED); dcur = np.zeros(EMBED)
    for j in range(T):                    # segment t = j+1
        ccur = MLs @ ccur + cLs
        Mcur = MLs @ Mcur
        dcur = MHs @ dcur + cHs
        Pcur = MHs @ Pcur
        base = j * BLK
        stack[base:base + EMBED] = Mcur.T.astype(np.float32)
        stack[base + EMBED:base + 2 * EMBED] = Pcur.T.astype(np.float32)
        stack[base + 2 * EMBED] = ccur.astype(np.float32)
        stack[base + 2 * EMBED + 1] = dcur.astype(np.float32)
        GT[:, j] = (Pcur.T @ q_w64[0]).astype(np.float32)
        GT[:, T + j] = (Pcur.T @ q_w64[1]).astype(np.float32)
        grow[0, j] = np.float32(q_w64[0] @ dcur + q_b64[0])
        grow[0, T + j] = np.float32(q_w64[1] @ dcur + q_b64[1])

    ident = np.eye(128, dtype=np.float32)
    ones_row = np.ones((1, 128), np.float32)
    ones_col = np.ones((128, 1), np.float32)
    p = np.arange(128, dtype=np.float32)[:, None]
    iota6 = np.concatenate(
        [p, p + 128, p + 256, p + 384,
         np.full((128, 1), 2 * EMBED, np.float32),
         np.full((128, 1), 2 * EMBED + 1, np.float32)], axis=1)
    maskmin = np.ones((T, 1), np.float32); maskmin[0, 0] = 0.0
    masklast = np.zeros((T, 1), np.float32); masklast[T - 1, 0] = 1.0
    tst = np.triu(np.ones((T, T), np.float32), 1)   # tst[k,m]=1 iff k<m
    tvec = np.arange(T, dtype=np.float32)[:, None]
    return dict(stack=stack, gt=GT, grow=grow, ident=ident,
                ones_row=ones_row, ones_col=ones_col, iota6=iota6,
                maskmin=maskmin, masklast=masklast, tst=tst, tvec=tvec)


DEBUG_BUILD = False


def _build_module():
    import concourse.bass as bass
    import concourse.mybir as mybir
    import concourse.tile as tile
    from concourse import bacc

    f32 = mybir.dt.float32
    i32 = mybir.dt.int32
    u8 = mybir.dt.uint8
    Alu = mybir.AluOpType
    Act = mybir.ActivationFunctionType

    nc = bacc.Bacc("TRN2", target_bir_lowering=False, debug=False,
                   enable_asserts=False, num_devices=N_CORES)

    # I/O
    czl = nc.dram_tensor("czl", [BP, EMBED], f32, kind="ExternalInput").ap()
    czh = nc.dram_tensor("czh", [BP, EMBED], f32, kind="ExternalInput").ap()
    ids = nc.dram_tensor("ids", [128, RT], i32, kind="ExternalInput").ap()
    dns = nc.dram_tensor("dns", [128, RT], u8, kind="ExternalInput").ap()
    trc = nc.dram_tensor("trc", [128, RT], u8, kind="ExternalInput").ap()
    stack = nc.dram_tensor("stack", [T * BLK, EMBED], f32, kind="ExternalInput").ap()
    gt = nc.dram_tensor("gt", [EMBED, 2 * T], f32, kind="ExternalInput").ap()
    grow = nc.dram_tensor("grow", [1, 2 * T], f32, kind="ExternalInput").ap()
    ident = nc.dram_tensor("ident", [128, 128], f32, kind="ExternalInput").ap()
    ones_row = nc.dram_tensor("ones_row", [1, 128], f32, kind="ExternalInput").ap()
    ones_col = nc.dram_tensor("ones_col", [128, 1], f32, kind="ExternalInput").ap()
    iota6 = nc.dram_tensor("iota6", [128, 6], f32, kind="ExternalInput").ap()
    maskmin = nc.dram_tensor("maskmin", [T, 1], f32, kind="ExternalInput").ap()
    masklast = nc.dram_tensor("masklast", [T, 1], f32, kind="ExternalInput").ap()
    tst = nc.dram_tensor("tst", [T, T], f32, kind="ExternalInput").ap()
    tvec = nc.dram_tensor("tvec", [T, 1], f32, kind="ExternalInput").ap()
    zl_out = nc.dram_tensor("zl_out", [BP, EMBED], f32, kind="ExternalOutput").ap()
    zh_out = nc.dram_tensor("zh_out", [BP, EMBED], f32, kind="ExternalOutput").ap()
    dbg = {}
    if DEBUG_BUILD:
        for dname, dshape in (("dbg_zg", [128, RT * EMBED]),
                              ("dbg_z0t", [128, BP]),
                              ("dbg_dq", [128, T]),
                              ("dbg_D", [T, 1]),
                              ("dbg_agr", [N_CORES, T]),
                              ("dbg_w", [T, 1]),
                              ("dbg_m", [1, 1]),
                              ("dbg_off", [128, 6]),
                              ("dbg_msel", [128, 6 * EMBED])):
            dbg[dname] = nc.dram_tensor(dname, dshape, f32,
                                        kind="ExternalOutput").ap()

    from contextlib import ExitStack
    with tile.TileContext(nc) as tc, ExitStack() as ctx:
        sb = ctx.enter_context(tc.tile_pool(name="sb", bufs=1))
        ps_mm = ctx.enter_context(tc.tile_pool(name="ps_mm", bufs=4, space="PSUM"))
        ps_q = ctx.enter_context(tc.tile_pool(name="ps_q", bufs=2, space="PSUM"))
        ps_d = ctx.enter_context(tc.tile_pool(name="ps_d", bufs=1, space="PSUM"))
        ps_s = ctx.enter_context(tc.tile_pool(name="ps_s", bufs=1, space="PSUM"))
        dram = ctx.enter_context(tc.tile_pool(name="dram", bufs=1, space="DRAM"))

        # ---- constants to SBUF ----
        ident_sb = sb.tile([128, 128], f32, tag="ident")
        nc.sync.dma_start(ident_sb[:], ident)
        # gt is [256, 22]: doesn't fit 128 partitions; store as two chunks
        gt0_sb = sb.tile([128, 2 * T], f32, tag="gt0")
        gt1_sb = sb.tile([128, 2 * T], f32, tag="gt1")
        nc.sync.dma_start(gt0_sb[:], gt[0:128, :])
        nc.sync.dma_start(gt1_sb[:], gt[128:256, :])
        grow_sb = sb.tile([1, 2 * T], f32, tag="grow")
        nc.sync.dma_start(grow_sb[:], grow)
        onesr_sb = sb.tile([1, 128], f32, tag="onesr")
        nc.sync.dma_start(onesr_sb[:], ones_row)
        onesc_sb = sb.tile([128, 1], f32, tag="onesc")
        nc.sync.dma_start(onesc_sb[:], ones_col)
        iota6_sb = sb.tile([128, 6], f32, tag="iota6")
        nc.sync.dma_start(iota6_sb[:], iota6)
        mmin_sb = sb.tile([T, 1], f32, tag="mmin")
        nc.sync.dma_start(mmin_sb[:], maskmin)
        mlast_sb = sb.tile([T, 1], f32, tag="mlast")
        nc.sync.dma_start(mlast_sb[:], masklast)
        tst_sb = sb.tile([T, T], f32, tag="tst")
        nc.sync.dma_start(tst_sb[:], tst)
        tvec_sb = sb.tile([T, 1], f32, tag="tvec")
        nc.sync.dma_start(tvec_sb[:], tvec)

        # ---- gather carries by local env ids; build keep mask ----
        ids_sb = sb.tile([128, RT], i32, tag="ids")
        nc.sync.dma_start(ids_sb[:], ids)
        d_sb = sb.tile([128, RT], u8, tag="d8")
        t_sb = sb.tile([128, RT], u8, tag="t8")
        nc.sync.dma_start(d_sb[:], dns)
        nc.sync.dma_start(t_sb[:], trc)
        rst_sb = sb.tile([128, RT], u8, tag="rst")
        nc.vector.tensor_tensor(out=rst_sb[:], in0=d_sb[:], in1=t_sb[:],
                                op=Alu.logical_or)
        rst_f = sb.tile([128, RT], f32, tag="rstf")
        nc.vector.tensor_copy(out=rst_f[:], in_=rst_sb[:])
        keep_f = sb.tile([128, RT], f32, tag="keepf")
        nc.vector.tensor_scalar(out=keep_f[:], in0=rst_f[:], scalar1=-1.0,
                                scalar2=1.0, op0=Alu.mult, op1=Alu.add)

        # zg[c][p, r*256 + j] = carry_c[ids[p, r], j]  (batch row = r*128+p)
        # NOTE: multi-offset-per-partition indirect DMA works in CoreSim but
        # silently writes nothing on HW — use one [128,1]-offset gather per
        # row-tile.
        zg = {}
        for cname, src in (("h", czh), ("l", czl)):
            zgt = sb.tile([128, RT * EMBED], f32, tag=f"zg_{cname}",
                          name=f"zg_{cname}")
            for r in range(RT):
                nc.gpsimd.indirect_dma_start(
                    out=zgt[:, r * EMBED:(r + 1) * EMBED], out_offset=None,
                    in_=src,
                    in_offset=bass.IndirectOffsetOnAxis(
                        ap=ids_sb[:, r:r + 1], axis=0))
            zg[cname] = zgt
        # mask resets: multiply row-tile r by keep column r
        for cname in ("h", "l"):
            for r in range(RT):
                nc.vector.tensor_scalar(
                    out=zg[cname][:, r * EMBED:(r + 1) * EMBED],
                    in0=zg[cname][:, r * EMBED:(r + 1) * EMBED],
                    scalar1=keep_f[:, r:r + 1], scalar2=None, op0=Alu.mult)
        if DEBUG_BUILD:
            nc.sync.dma_start(dbg["dbg_zg"], zg["h"][:])

        # ---- transpose z0 into [D, rows] layout (H first: critical path) ----
        z0T = {}
        for cname in ("h", "l"):
            for k in range(2):
                z0T[cname, k] = sb.tile([128, BP], f32, tag=f"z0T_{cname}{k}",
                                        name=f"z0T_{cname}{k}")
        for cname in ("h", "l"):
            for r in range(RT):
                for k in range(2):
                    pst = ps_mm.tile([128, 128], f32, tag="mm")
                    nc.tensor.transpose(
                        pst[:],
                        zg[cname][:, r * EMBED + k * 128: r * EMBED + (k + 1) * 128],
                        ident_sb[:])
                    nc.vector.tensor_copy(
                        out=z0T[cname, k][:, r * 128:(r + 1) * 128], in_=pst[:])

        if DEBUG_BUILD:
            nc.sync.dma_start(dbg["dbg_z0t"], z0T["h", 0][:])
        # ---- q logits for all 11 segments + partial sums ----
        dq = []
        for r in range(RT):
            qps = ps_q.tile([128, 2 * T], f32, tag="qps")
            nc.tensor.matmul(qps[:], z0T["h", 0][:, r * 128:(r + 1) * 128],
                             gt0_sb[:], start=True, stop=False)
            nc.tensor.matmul(qps[:], z0T["h", 1][:, r * 128:(r + 1) * 128],
                             gt1_sb[:], start=False, stop=False)
            nc.tensor.matmul(qps[:], onesr_sb[:], grow_sb[:],
                             start=False, stop=True)
            sig = sb.tile([128, 2 * T], f32, tag="sig", bufs=2)
            nc.scalar.activation(sig[:], qps[:], Act.Sigmoid)
            dqt = sb.tile([128, T], f32, tag=f"dq{r}")
            nc.vector.tensor_tensor(out=dqt[:], in0=sig[:, 0:T],
                                    in1=sig[:, T:2 * T], op=Alu.subtract)
            dq.append(dqt)
            if DEBUG_BUILD and r == 0:
                nc.sync.dma_start(dbg["dbg_dq"], dqt[:])
        Dps = ps_d.tile([T, 1], f32, tag="Dps")
        for r in range(RT):
            nc.tensor.matmul(Dps[:], dq[r][:], onesc_sb[:],
                             start=(r == 0), stop=(r == RT - 1))
        D_sb = sb.tile([T, 1], f32, tag="Dsb")
        nc.vector.tensor_copy(out=D_sb[:], in_=Dps[:])

        if DEBUG_BUILD:
            nc.sync.dma_start(dbg["dbg_D"], D_sb[:])
        # ---- one AllGather of the 11 partial sums ----
        ag_in = dram.tile([T, 1], f32, tag="ag_in")
        ag_out = dram.tile([N_CORES * T, 1], f32, tag="ag_out",
                           addr_space="Shared")
        nc.sync.dma_start(ag_in[:], D_sb[:])
        nc.gpsimd.collective_compute(
            "AllGather", Alu.bypass,
            replica_groups=[list(range(N_CORES))],
            ins=[ag_in.opt()], outs=[ag_out.opt()])
        agr_sb = sb.tile([N_CORES, T], f32, tag="agr")
        nc.sync.dma_start(agr_sb[:],
                          ag_out.rearrange("(a b) c -> a (b c)", a=N_CORES))

        if DEBUG_BUILD:
            nc.sync.dma_start(dbg["dbg_agr"], agr_sb[:])
        # ---- halting logic: find first segment with halt ----
        Dg_ps = ps_s.tile([T, 1], f32, tag="t")
        nc.tensor.matmul(Dg_ps[:], agr_sb[:], onesc_sb[0:N_CORES, :],
                         start=True, stop=True)
        h_sb = sb.tile([T, 1], f32, tag="h1")
        nc.vector.tensor_scalar(out=h_sb[:], in0=Dg_ps[:], scalar1=0.0,
                                scalar2=None, op0=Alu.is_gt)
        nc.vector.tensor_tensor(out=h_sb[:], in0=h_sb[:], in1=mmin_sb[:],
                                op=Alu.mult)
        nc.vector.tensor_tensor(out=h_sb[:], in0=h_sb[:], in1=mlast_sb[:],
                                op=Alu.max)
        cps = ps_s.tile([T, 1], f32, tag="t")
        nc.tensor.matmul(cps[:], tst_sb[:], h_sb[:], start=True, stop=True)
        notc = sb.tile([T, 1], f32, tag="notc")
        nc.vector.tensor_scalar(out=notc[:], in0=cps[:], scalar1=-1.0,
                                scalar2=1.0, op0=Alu.mult, op1=Alu.add)
        nc.vector.tensor_scalar(out=notc[:], in0=notc[:], scalar1=0.0,
                                scalar2=None, op0=Alu.max)
        w_sb = sb.tile([T, 1], f32, tag="wsb")
        nc.vector.tensor_tensor(out=w_sb[:], in0=h_sb[:], in1=notc[:],
                                op=Alu.mult)
        if DEBUG_BUILD:
            nc.sync.dma_start(dbg["dbg_w"], w_sb[:])
        mps = ps_s.tile([1, 1], f32, tag="t")
        nc.tensor.matmul(mps[:], w_sb[:], tvec_sb[:], start=True, stop=True)
        m_sb = sb.tile([1, 1], f32, tag="msb")
        nc.vector.tensor_copy(out=m_sb[:], in_=mps[:])
        if DEBUG_BUILD:
            nc.sync.dma_start(dbg["dbg_m"], m_sb[:])
        bps = ps_s.tile([128, 1], f32, tag="t")
        nc.tensor.matmul(bps[:], onesr_sb[:], m_sb[:], start=True, stop=True)
        m514 = sb.tile([128, 1], f32, tag="m514")
        nc.scalar.mul(m514[:], bps[:], float(BLK))
        off_f = sb.tile([128, 6], f32, tag="offf")
        nc.vector.tensor_scalar(out=off_f[:], in0=iota6_sb[:],
                                scalar1=m514[:], scalar2=None, op0=Alu.add)
        if DEBUG_BUILD:
            nc.sync.dma_start(dbg["dbg_off"], off_f[:])
        off_i = sb.tile([128, 6], i32, tag="offi")
        nc.vector.tensor_copy(out=off_i[:], in_=off_f[:])

        # ---- gather the selected segment's affine maps ----
        msel = sb.tile([128, 6 * EMBED], f32, tag="msel")
        for j in range(6):
            nc.gpsimd.indirect_dma_start(
                out=msel[:, j * EMBED:(j + 1) * EMBED], out_offset=None,
                in_=stack,
                in_offset=bass.IndirectOffsetOnAxis(
                    ap=off_i[:, j:j + 1], axis=0))

        if DEBUG_BUILD:
            nc.sync.dma_start(dbg["dbg_msel"], msel[:])
        # ---- final states: z_final = z0 @ M_m.T + c_m (row-major out) ----
        for ci, (cname, outdram) in enumerate((("h", zh_out), ("l", zl_out))):
            mbase = (2 * EMBED) if cname == "h" else 0
            bcol = (5 * EMBED) if cname == "h" else (4 * EMBED)
            for r in range(RT):
                fps = ps_mm.tile([128, EMBED], f32, tag="mm")
                nc.tensor.matmul(fps[:], z0T[cname, 0][:, r * 128:(r + 1) * 128],
                                 msel[:, mbase:mbase + EMBED],
                                 start=True, stop=False)
                nc.tensor.matmul(fps[:], z0T[cname, 1][:, r * 128:(r + 1) * 128],
                                 msel[:, mbase + EMBED:mbase + 2 * EMBED],
                                 start=False, stop=False)
                nc.tensor.matmul(fps[:], onesr_sb[:],
                                 msel[0:1, bcol:bcol + EMBED],
                                 start=False, stop=True)
                osb = sb.tile([128, EMBED], f32, tag="osb", bufs=4)
                nc.vector.tensor_copy(out=osb[:], in_=fps[:])
                nc.sync.dma_start(outdram[r * 128:(r + 1) * 128, :], osb[:])

    nc.compile()
    return nc


_CACHE = {}


def _get_module():
    if "nc" not in _CACHE:
        _CACHE["nc"] = _build_module()
    return _CACHE["nc"]


TRACE = False
LAST_RESULTS = None


def kernel(x, carry_z_l, carry_z_h, L_w, L_b, H_w, H_b, q_w, q_b,
           training_env_ids, dones, truncateds):
    global LAST_RESULTS
    from concourse.bass_utils import run_bass_kernel_spmd

    carry_z_l = np.ascontiguousarray(np.asarray(carry_z_l, np.float32))
    carry_z_h = np.ascontiguousarray(np.asarray(carry_z_h, np.float32))
    ids_full = np.asarray(training_env_ids, np.int32)
    dones = np.asarray(dones).astype(np.uint8)
    truncateds = np.asarray(truncateds).astype(np.uint8)

    consts = _host_consts(np.asarray(L_w, np.float32), np.asarray(L_b, np.float32),
                          np.asarray(H_w, np.float32), np.asarray(H_b, np.float32),
                          np.asarray(q_w, np.float32), np.asarray(q_b, np.float32))
    shared = {k: np.ascontiguousarray(v) for k, v in consts.items()}

    in_maps = []
    for c in range(N_CORES):
        sl = slice(c * BP, (c + 1) * BP)
        ids_loc = (ids_full[sl] - c * BP).astype(np.int32)
        m = dict(shared)
        m["czl"] = carry_z_l[sl]
        m["czh"] = carry_z_h[sl]
        # [128, RT] with element (p, r) = batch row r*128+p of this core
        m["ids"] = np.ascontiguousarray(ids_loc.reshape(RT, 128).T)
        m["dns"] = np.ascontiguousarray(dones[sl].reshape(RT, 128).T)
        m["trc"] = np.ascontiguousarray(truncateds[sl].reshape(RT, 128).T)
        in_maps.append(m)

    nc = _get_module()
    res = run_bass_kernel_spmd(nc, in_maps, core_ids=list(range(N_CORES)),
                               trace=TRACE)
    LAST_RESULTS = res

    zl_full = np.concatenate([res.results[c]["zl_out"] for c in range(N_CORES)], 0)
    zh_full = np.concatenate([res.results[c]["zh_out"] for c in range(N_CORES)], 0)

    new_czl = carry_z_l.copy()
    new_czh = carry_z_h.copy()
    new_czl[ids_full] = zl_full
    new_czh[ids_full] = zh_full
    return zh_full, new_czl, new_czh


# revision 13
# speedup vs baseline: 2.0817x; 2.0817x over previous
"""Trainium2 Bass kernel for nn_HRMReasoning (8-core data parallel).

Key math: stack_pass is affine (z -> z @ W.T + b composed 6x), so every
segment's L-part (15 stack passes) and H-part (3 stack passes) collapse to
single affine maps; segment t's cumulative map is the t-th power. The ACT
halting trajectory only needs q_t = sigmoid(zh_t @ q_w.T + q_b) where
zh_t = zh_0 @ (P^t).T + d_t, so all 11 segment logits come from ONE matmul
against a folded [256, 22] matrix. The final state is selected by the
halting index m via an indirect-DMA gather from a precomposed power table,
then applied with 2 accumulating matmuls per output tile.

Communication-avoiding halting: instead of an all-reduce per segment (or
even one all-gather), EVERY core evaluates the q partial sums over the
full 4096-row batch (16 matmuls) — all cores run the same arithmetic on
the same replicated activations, so they reach bitwise-identical halting
decisions with zero cross-core communication. On this harness the 8 core
launches are staggered by tens of microseconds, so any collective stalls
every core for the full skew; redundant compute is ~7us and fully local.

Sharding: batch dim block-sharded across 8 cores. The env-id gather /
reset masking / final scatter are data movement done host-side during
shard prep and unshard (general: any ids, dones, truncateds).
"""

import numpy as np

EMBED = 256
NUM_LAYERS = 6
H_CYCLES = 3
L_CYCLES = 5
MMIN = 1
MMAX = 10
T = MMAX + 1          # 11 segments max
B = 4096
N_CORES = 8
BP = B // N_CORES     # 512 rows per core
RT = BP // 128        # 4 row-tiles per core
BLK = EMBED + 1       # 257 rows per segment block in the power table
NCH = B // 512        # 8 n-chunks for the replicated q evaluation

# q logits live on partitions 0:11 (q0) and 32:43 (q1) — partition slices
# must start at multiples of 32 on TRN2.
QW = 64           # q-logit partition width (one-hot padded)
Q1 = 32           # base partition of the q1 block
# constpack column layout ([128, CP_W] f32)
C_GT0 = 0         # [:, 0:64]    padded GT rows 0:128
C_GT1 = 64        # [:, 64:128]  padded GT rows 128:256
C_GROW = 128      # [0:64, 128]  q bias (padded column)
C_MMIN = 129      # [0:11, 129]
C_MLAST = 130     # [0:11, 130]
C_TST = 131       # [0:11, 131:142]
C_TVEC = 142      # [0:11, 142]
C_IOTA = 143      # [:, 143:145]  [p, 128+p]
C_ONESR = 145     # [0, 145:273]  row of 128 ones
C_SEL = 273       # [0:64, 273:284] +-1 q-sum selection (D = sel.T @ ssum)
CP_W = 288


def _compose_stack(W, bvec):
    """Affine map M, c with stack_pass(z) == z @ M.T + c (float64)."""
    M = np.eye(EMBED, dtype=np.float64)
    c = np.zeros(EMBED, dtype=np.float64)
    for i in range(NUM_LAYERS):
        Wi = W[i].astype(np.float64)
        M = Wi @ M
        c = Wi @ c + bvec[i].astype(np.float64)
    return M, c


def _compose_pow(M, c, n):
    Mn = np.eye(EMBED, dtype=np.float64)
    cn = np.zeros(EMBED, dtype=np.float64)
    for _ in range(n):
        cn = M @ cn + c
        Mn = M @ Mn
    return Mn, cn


def _host_consts(L_w, L_b, H_w, H_b, q_w, q_b):
    ML, cL = _compose_stack(L_w, L_b)
    MH, cH = _compose_stack(H_w, H_b)
    MLs, cLs = _compose_pow(ML, cL, 15)   # one segment of L
    MHs, cHs = _compose_pow(MH, cH, 3)    # one segment of H

    q_w64 = q_w.astype(np.float64)
    q_b64 = q_b.astype(np.float64)

    # stack2[t*257 + k, :] = [ML^t.T[k, :], MH^t.T[k, :]] for k < 256
    # stack2[t*257 + 256, :] = [cL_t, cH_t]
    stack2 = np.zeros((T * BLK, 2 * EMBED), np.float32)
    GT = np.zeros((EMBED, 2 * T), np.float32)
    growT = np.zeros(2 * T, np.float32)

    Mcur = np.eye(EMBED); ccur = np.zeros(EMBED)
    Pcur = np.eye(EMBED); dcur = np.zeros(EMBED)
    for j in range(T):                    # segment t = j+1
        ccur = MLs @ ccur + cLs
        Mcur = MLs @ Mcur
        dcur = MHs @ dcur + cHs
        Pcur = MHs @ Pcur
        base = j * BLK
        stack2[base:base + EMBED, 0:EMBED] = Mcur.T.astype(np.float32)
        stack2[base:base + EMBED, EMBED:] = Pcur.T.astype(np.float32)
        stack2[base + EMBED, 0:EMBED] = ccur.astype(np.float32)
        stack2[base + EMBED, EMBED:] = dcur.astype(np.float32)
        GT[:, j] = (Pcur.T @ q_w64[0]).astype(np.float32)
        GT[:, T + j] = (Pcur.T @ q_w64[1]).astype(np.float32)
        growT[j] = np.float32(q_w64[0] @ dcur + q_b64[0])
        growT[T + j] = np.float32(q_w64[1] @ dcur + q_b64[1])

    cp = np.zeros((128, CP_W), np.float32)
    cp[:, C_GT0:C_GT0 + T] = GT[0:128, 0:T]
    cp[:, C_GT0 + Q1:C_GT0 + Q1 + T] = GT[0:128, T:2 * T]
    cp[:, C_GT1:C_GT1 + T] = GT[128:256, 0:T]
    cp[:, C_GT1 + Q1:C_GT1 + Q1 + T] = GT[128:256, T:2 * T]
    cp[0:T, C_GROW] = growT[0:T]
    cp[Q1:Q1 + T, C_GROW] = growT[T:2 * T]
    cp[0:T, C_MMIN] = 1.0; cp[0, C_MMIN] = 0.0
    cp[T - 1, C_MLAST] = 1.0
    cp[0:T, C_TST:C_TST + T] = np.triu(np.ones((T, T), np.float32), 1)
    cp[0:T, C_TVEC] = np.arange(T, dtype=np.float32)
    cp[:, C_IOTA] = np.arange(128, dtype=np.float32)
    cp[:, C_IOTA + 1] = np.arange(128, dtype=np.float32) + 128.0
    cp[0, C_ONESR:C_ONESR + 128] = 1.0
    for t in range(T):
        cp[t, C_SEL + t] = 1.0
        cp[Q1 + t, C_SEL + t] = -1.0
    return dict(stack2=stack2, cpk=cp)


def _build_module():
    import concourse.bass as bass
    import concourse.mybir as mybir
    import concourse.tile as tile
    from concourse import bacc
    from contextlib import ExitStack

    f32 = mybir.dt.float32
    i32 = mybir.dt.int32
    Alu = mybir.AluOpType
    Act = mybir.ActivationFunctionType

    nc = bacc.Bacc("TRN2", target_bir_lowering=False, debug=False,
                   enable_asserts=False, num_devices=N_CORES)

    # I/O.  zfhT: full-batch masked-gathered z_h, transposed [256, 4096]
    #       (replicated to every core for the local halting decision).
    #       zslT/zshT: this core's own 512-column slice of z_l / z_h.
    zfhT = nc.dram_tensor("zfhT", [EMBED, B], f32, kind="ExternalInput").ap()
    zslT = nc.dram_tensor("zslT", [EMBED, BP], f32, kind="ExternalInput").ap()
    zshT = nc.dram_tensor("zshT", [EMBED, BP], f32, kind="ExternalInput").ap()
    stack2 = nc.dram_tensor("stack2", [T * BLK, 2 * EMBED], f32,
                            kind="ExternalInput").ap()
    cpk = nc.dram_tensor("cpk", [128, CP_W], f32, kind="ExternalInput").ap()
    zl_out = nc.dram_tensor("zl_out", [BP, EMBED], f32, kind="ExternalOutput").ap()
    zh_out = nc.dram_tensor("zh_out", [BP, EMBED], f32, kind="ExternalOutput").ap()

    with tile.TileContext(nc) as tc, ExitStack() as ctx:
        sb = ctx.enter_context(tc.tile_pool(name="sb", bufs=1))
        ps_q = ctx.enter_context(tc.tile_pool(name="ps_q", bufs=3, space="PSUM"))
        ps_f = ctx.enter_context(tc.tile_pool(name="ps_f", bufs=4, space="PSUM"))
        ps_s = ctx.enter_context(tc.tile_pool(name="ps_s", bufs=1, space="PSUM"))

        cp = sb.tile([128, CP_W], f32, tag="cp")
        nc.sync.dma_start(cp[:], cpk)
        gt0 = cp[:, C_GT0:C_GT0 + QW]
        gt1 = cp[:, C_GT1:C_GT1 + QW]
        onesr = cp[0:1, C_ONESR:C_ONESR + 128]

        # own-slice activations (stationaries for the final matmuls)
        zown = {}
        for cname, src in (("l", zslT), ("h", zshT)):
            for k in range(2):
                zt = sb.tile([128, BP], f32, tag=f"zown_{cname}{k}",
                             name=f"zown_{cname}{k}")
                nc.scalar.dma_start(zt[:], src[k * 128:(k + 1) * 128, :])
                zown[cname, k] = zt

        # ---- replicated q: logits for all 11 segments over all 4096 rows ----
        # sigmoid row-sums accumulate during the activation (accum_out);
        # D_t = (sum sig0) - (sum sig1) falls out of one +-1 matmul.
        ssum8 = sb.tile([QW, NCH], f32, tag="ssum8")
        for c in range(NCH):
            qps = ps_q.tile([QW, 512], f32, tag="qps")
            for k in range(2):
                qr = sb.tile([128, 512], f32, tag="qr", bufs=6)
                nc.sync.dma_start(qr[:], zfhT[k * 128:(k + 1) * 128,
                                              c * 512:(c + 1) * 512])
                nc.tensor.matmul(qps[:], gt0 if k == 0 else gt1, qr[:],
                                 start=(k == 0), stop=(k == 1))
            sig = sb.tile([QW, 512], f32, tag="sig", bufs=2)
            nc.scalar.activation(sig[:], qps[:], Act.Sigmoid,
                                 bias=cp[0:QW, C_GROW:C_GROW + 1],
                                 accum_out=ssum8[:, c:c + 1])
        ssum = sb.tile([QW, 1], f32, tag="ssum")
        nc.vector.reduce_sum(out=ssum[:], in_=ssum8[:],
                             axis=mybir.AxisListType.X)
        Dps = ps_s.tile([T, 1], f32, tag="t")
        nc.tensor.matmul(Dps[:], cp[0:QW, C_SEL:C_SEL + T], ssum[:],
                         start=True, stop=True)

        # ---- halting: first t>=2 with sum0>sum1, else t=11 (one-hot w) ----
        h_sb = sb.tile([T, 1], f32, tag="h1")
        nc.vector.tensor_scalar(out=h_sb[:], in0=Dps[:], scalar1=0.0,
                                scalar2=None, op0=Alu.is_gt)
        nc.vector.tensor_tensor(out=h_sb[:], in0=h_sb[:],
                                in1=cp[0:T, C_MMIN:C_MMIN + 1], op=Alu.mult)
        nc.vector.tensor_tensor(out=h_sb[:], in0=h_sb[:],
                                in1=cp[0:T, C_MLAST:C_MLAST + 1], op=Alu.max)
        cps = ps_s.tile([T, 1], f32, tag="t")
        nc.tensor.matmul(cps[:], cp[0:T, C_TST:C_TST + T], h_sb[:],
                         start=True, stop=True)
        notc = sb.tile([T, 1], f32, tag="notc")
        nc.vector.tensor_scalar(out=notc[:], in0=cps[:], scalar1=-1.0,
                                scalar2=1.0, op0=Alu.mult, op1=Alu.add)
        nc.vector.tensor_scalar(out=notc[:], in0=notc[:], scalar1=0.0,
                                scalar2=None, op0=Alu.max)
        w_sb = sb.tile([T, 1], f32, tag="wsb")
        nc.vector.tensor_tensor(out=w_sb[:], in0=h_sb[:], in1=notc[:],
                                op=Alu.mult)
        mps = ps_s.tile([1, 1], f32, tag="t")
        nc.tensor.matmul(mps[:], w_sb[:], cp[0:T, C_TVEC:C_TVEC + 1],
                         start=True, stop=True)
        m_sb = sb.tile([1, 1], f32, tag="msb")
        nc.vector.tensor_copy(out=m_sb[:], in_=mps[:])
        bps = ps_s.tile([128, 1], f32, tag="t")
        nc.tensor.matmul(bps[:], onesr, m_sb[:], start=True, stop=True)
        m257 = sb.tile([128, 1], f32, tag="m257")
        nc.scalar.mul(m257[:], bps[:], float(BLK))
        off_f = sb.tile([128, 2], f32, tag="offf")
        nc.vector.tensor_scalar(out=off_f[:], in0=cp[:, C_IOTA:C_IOTA + 2],
                                scalar1=m257[:], scalar2=None, op0=Alu.add)
        off_i = sb.tile([128, 2], i32, tag="offi")
        nc.vector.tensor_copy(out=off_i[:], in_=off_f[:])
        boff_f = sb.tile([2, 1], f32, tag="bofff")
        nc.vector.tensor_scalar(out=boff_f[:], in0=m257[0:2, :],
                                scalar1=float(EMBED), scalar2=None, op0=Alu.add)
        boff_i = sb.tile([2, 1], i32, tag="boffi")
        nc.vector.tensor_copy(out=boff_i[:], in_=boff_f[:])

        # ---- gather the selected segment's [ML^m.T | MH^m.T] and biases ----
        msel = {}
        for k in range(2):
            mt = sb.tile([128, 2 * EMBED], f32, tag=f"msel{k}",
                         name=f"msel{k}")
            nc.gpsimd.indirect_dma_start(
                out=mt[:], out_offset=None, in_=stack2,
                in_offset=bass.IndirectOffsetOnAxis(ap=off_i[:, k:k + 1],
                                                    axis=0))
            msel[k] = mt
        mbias = sb.tile([2, 2 * EMBED], f32, tag="mbias")
        nc.gpsimd.indirect_dma_start(
            out=mbias[:], out_offset=None, in_=stack2,
            in_offset=bass.IndirectOffsetOnAxis(ap=boff_i[:], axis=0))

        # ---- final states: z = z0 @ M_m.T + c_m (row-major out) ----
        for cname, outdram in (("h", zh_out), ("l", zl_out)):
            mc = EMBED if cname == "h" else 0
            for r in range(RT):
                fps = ps_f.tile([128, EMBED], f32, tag="fps")
                nc.tensor.matmul(fps[:], zown[cname, 0][:, r * 128:(r + 1) * 128],
                                 msel[0][:, mc:mc + EMBED],
                                 start=True, stop=False)
                nc.tensor.matmul(fps[:], zown[cname, 1][:, r * 128:(r + 1) * 128],
                                 msel[1][:, mc:mc + EMBED],
                                 start=False, stop=False)
                nc.tensor.matmul(fps[:], onesr, mbias[0:1, mc:mc + EMBED],
                                 start=False, stop=True)
                osb = sb.tile([128, EMBED], f32, tag="osb", bufs=4)
                nc.vector.tensor_copy(out=osb[:], in_=fps[:])
                eng = nc.sync if r % 2 == 0 else nc.scalar
                eng.dma_start(outdram[r * 128:(r + 1) * 128, :], osb[:])

    nc.compile()
    return nc


_CACHE = {}


def _get_module():
    if "nc" not in _CACHE:
        _CACHE["nc"] = _build_module()
    return _CACHE["nc"]


TRACE = False
LAST_RESULTS = None


def _prep_inputs(carry_z_l, carry_z_h, ids_full, dones, truncateds, consts):
    """Shard prep: env-id gather + reset mask + feature-major transpose."""
    reset = (dones | truncateds).astype(bool)
    z0l = carry_z_l[ids_full]
    z0h = carry_z_h[ids_full]
    z0l[reset] = 0.0
    z0h[reset] = 0.0
    zflT = np.ascontiguousarray(z0l.T)
    zfhT = np.ascontiguousarray(z0h.T)
    in_maps = []
    for c in range(N_CORES):
        m = dict(consts)
        m["zfhT"] = zfhT
        m["zslT"] = np.ascontiguousarray(zflT[:, c * BP:(c + 1) * BP])
        m["zshT"] = np.ascontiguousarray(zfhT[:, c * BP:(c + 1) * BP])
        in_maps.append(m)
    return in_maps


def kernel(x, carry_z_l, carry_z_h, L_w, L_b, H_w, H_b, q_w, q_b,
           training_env_ids, dones, truncateds):
    global LAST_RESULTS
    from concourse.bass_utils import run_bass_kernel_spmd

    carry_z_l = np.ascontiguousarray(np.asarray(carry_z_l, np.float32))
    carry_z_h = np.ascontiguousarray(np.asarray(carry_z_h, np.float32))
    ids_full = np.asarray(training_env_ids, np.int32)
    dones = np.asarray(dones).astype(bool)
    truncateds = np.asarray(truncateds).astype(bool)

    consts = _host_consts(np.asarray(L_w, np.float32), np.asarray(L_b, np.float32),
                          np.asarray(H_w, np.float32), np.asarray(H_b, np.float32),
                          np.asarray(q_w, np.float32), np.asarray(q_b, np.float32))
    in_maps = _prep_inputs(carry_z_l, carry_z_h, ids_full, dones,
                           truncateds, consts)

    nc = _get_module()
    res = run_bass_kernel_spmd(nc, in_maps, core_ids=list(range(N_CORES)),
                               trace=TRACE)
    LAST_RESULTS = res

    zl_full = np.concatenate([res.results[c]["zl_out"] for c in range(N_CORES)], 0)
    zh_full = np.concatenate([res.results[c]["zh_out"] for c in range(N_CORES)], 0)

    new_czl = carry_z_l.copy()
    new_czh = carry_z_h.copy()
    new_czl[ids_full] = zl_full
    new_czh[ids_full] = zh_full
    return zh_full, new_czl, new_czh
